# revision 2
# baseline (speedup 1.0000x reference)
"""L1 loss (mean |yhat - y|) over (64, 128, 4096) fp32 tensors on 8 TRN2 cores.

Strategy: pure data-parallel over the batch dim. Core i takes batch rows
[8i, 8i+8) of both tensors, viewed as a flat 4,194,304-element chunk and
re-chunked as [128, 32768] (sum is permutation-invariant, so any fixed
bijection shared by yhat and y is valid — this makes host prep zero-copy).

Per tile t (free-dim slice of width F): the yhat half loads via the Sync
HWDGE ring and the y half via the Scalar HWDGE ring, so both hardware
descriptor queues stay full (2x outstanding descriptors vs one ring —
hides HBM read latency). The vector engine computes d = yhat - y, then an
abs-sum reduce into one column of a [128, n_tiles] accumulator. Tile sizes
taper (4096 ... 256) so the serial compute tail after the final DMA is
~2.5us instead of ~11us. Host sums the partials in float64.
"""

import numpy as np

import concourse.bacc as bacc
import concourse.bass as bass
import concourse.mybir as mybir
import concourse.tile as tile
from concourse.bass_utils import run_bass_kernel_spmd

N_CORES = 8
FULL_SHAPE = (64, 128, 4096)
TOTAL_ELEMS = FULL_SHAPE[0] * FULL_SHAPE[1] * FULL_SHAPE[2]  # 33,554,432

P = 128                                   # SBUF partitions
ELEMS_PER_CORE = TOTAL_ELEMS // N_CORES   # 4,194,304 per input tensor
F_TOTAL = ELEMS_PER_CORE // P             # 32,768 floats per partition

# Tapered tile widths: big tiles for steady-state DMA efficiency, small
# ones at the end so the last tile's sub+reduce tail is short.
F_TILES = [4096] * 7 + [2048, 1024, 512, 256, 256]
assert sum(F_TILES) == F_TOTAL
N_TILES = len(F_TILES)

_nc_cache = []


def _build_nc():
    # Bacc (not raw Bass): its compile() pipeline runs
    # generate_event_semaphores, which splits multi-wait sync_infos to
    # satisfy the TRN2 1-wait-per-instruction constraint walrus enforces.
    nc = bacc.Bacc("TRN2", target_bir_lowering=False, debug=False)
    yh = nc.declare_dram_parameter("yh", [P, F_TOTAL], mybir.dt.float32, isOutput=False)
    yy = nc.declare_dram_parameter("yy", [P, F_TOTAL], mybir.dt.float32, isOutput=False)
    out = nc.declare_dram_parameter("out", [P, N_TILES], mybir.dt.float32, isOutput=True)

    with tile.TileContext(nc) as tc:
        with (
            tc.tile_pool(name="ina", bufs=4) as a_pool,
            tc.tile_pool(name="inb", bufs=4) as b_pool,
            tc.tile_pool(name="diff", bufs=2) as diff_pool,
            tc.tile_pool(name="acc", bufs=1) as acc_pool,
        ):
            acc = acc_pool.tile([P, N_TILES], mybir.dt.float32)
            off = 0
            for i, f in enumerate(F_TILES):
                at = a_pool.tile([P, f], mybir.dt.float32, tag="a")
                bt = b_pool.tile([P, f], mybir.dt.float32, tag="b")
                # Two independent HWDGE rings: SP (sync) and ACT (scalar).
                nc.sync.dma_start(at[:], yh[:, off : off + f])
                nc.scalar.dma_start(bt[:], yy[:, off : off + f])
                d = diff_pool.tile([P, f], mybir.dt.float32, tag="d")
                nc.vector.tensor_sub(d[:], at[:], bt[:])
                nc.vector.tensor_reduce(
                    acc[:, i : i + 1],
                    d[:],
                    axis=mybir.AxisListType.X,
                    op=mybir.AluOpType.add,
                    apply_absolute_value=True,
                )
                off += f
            nc.sync.dma_start(out[:], acc[:])
    nc.compile()
    return nc


def _get_nc():
    if not _nc_cache:
        _nc_cache.append(_build_nc())
    return _nc_cache[0]


def _shard_inputs(yhat: np.ndarray, y: np.ndarray) -> list[dict[str, np.ndarray]]:
    yh = np.ascontiguousarray(yhat, dtype=np.float32).reshape(N_CORES, P, F_TOTAL)
    yy = np.ascontiguousarray(y, dtype=np.float32).reshape(N_CORES, P, F_TOTAL)
    return [{"yh": yh[c], "yy": yy[c]} for c in range(N_CORES)]


def kernel(yhat: np.ndarray, y: np.ndarray) -> np.ndarray:
    nc = _get_nc()
    in_maps = _shard_inputs(yhat, y)
    res = run_bass_kernel_spmd(nc, in_maps, list(range(N_CORES)))
    total = np.float64(0.0)
    for r in res.results:
        total += r["out"].astype(np.float64).sum()
    return np.asarray(total / TOTAL_ELEMS, dtype=np.float32)
